# revision 1
# baseline (speedup 1.0000x reference)
"""Trainium2 Bass kernel for CycleBalanceLoss.

loss = ALPHA * mean_b |sum_l adj[b, argmax_l, argmax_{l+1}]|
     + (1-ALPHA) * mean_{b,l} (logsumexp(logits[b,l,:]) - logits[b,l,t[b,l]])

Sharding: pure data parallel over the batch dim B=64 across 8 cores
(BPC=8 batches per core). Host sums the 8 per-core partial scalars.

Per core:
  - stream the logits shard [8, 128, 1024] f32 through SBUF;
  - ScalarE computes exp(x) with a per-row f32 accumulator (-> logsumexp),
    writing the exp values as fp16: argmax(exp(x)) == argmax(x), so the
    DVE max/max_index pass runs on 2-byte data;
  - target logits are gathered with per-column HW-DGE indirect DMAs
    (host-precomputed flat indices) issued before the loop so they overlap
    the stream;
  - the adjacency pair index pair[l] = 1024*idx[l] + idx[l+1] is ONE tiny
    PE matmul per batch against PAIRM = 1024*I + subdiag(1) (the subdiag
    also does the l+1 partition shift and zeroes the pad row), then a
    cast to u32 feeds a per-batch indirect DMA issued right after that
    batch's argmax, so gathers pace with the loop and only the last
    batch's gather (~2.7us + transfer) sits in the tail;
  - tc.tile_wait_until phases pin per-engine queue order so the tile
    scheduler cannot hoist gather-dependent ops ahead of the argmax work
    (its DMA cost model is optimistic, which otherwise stalls the
    in-order queues).
"""

import numpy as np

B, L, N = 64, 128, 1024
NCORES = 8
BPC = B // NCORES
ALPHA = 0.7

_CACHE = {}


def _build():
    import concourse.bacc as bacc
    import concourse.tile as tile
    from concourse import bass, mybir

    f32 = mybir.dt.float32
    fp16 = mybir.dt.float16
    i32 = mybir.dt.int32
    u16 = mybir.dt.uint16
    u32 = mybir.dt.uint32
    AF = mybir.ActivationFunctionType
    Alu = mybir.AluOpType
    AX = mybir.AxisListType

    nc = bacc.Bacc(
        "TRN2",
        target_bir_lowering=False,
        debug=False,
        num_devices=NCORES,
    )

    logits = nc.dram_tensor("logits", [BPC, L, N], f32, kind="ExternalInput")
    tfidx = nc.dram_tensor("tfidx", [L, BPC], i32, kind="ExternalInput")
    adjt = [
        nc.dram_tensor(f"adj{b}", [N * N, 1], f32, kind="ExternalInput")
        for b in range(BPC)
    ]
    pairm = nc.dram_tensor("pairm", [128, 128], f32, kind="ExternalInput")
    out = nc.dram_tensor("out", [2, 1], f32, kind="ExternalOutput")

    logits_ap = logits.ap()
    logits_flat = logits_ap.rearrange("b l n -> (b l n)")[:, None]

    with tile.TileContext(nc) as tc:
        with (
            tc.tile_pool(name="xp", bufs=4) as xp,
            tc.tile_pool(name="ep", bufs=3) as ep,
            tc.tile_pool(name="sp", bufs=3) as sp,
            tc.tile_pool(name="acc", bufs=1) as accp,
            tc.tile_pool(name="psum", bufs=2, space="PSUM") as pp,
        ):
            ones = accp.tile([L, 1], f32)
            nc.vector.memset(ones[:], 1.0)

            PM = accp.tile([128, 128], f32)
            nc.scalar.dma_start(PM[:], pairm.ap())

            # target flat indices (host precomputed) and target-logit gather.
            # NOTE: multi-column offset tables wedge the HW DGE - one
            # indirect DMA per column ([P,1] offsets) is the proven shape.
            TF = accp.tile([L, BPC], i32)
            nc.scalar.dma_start(TF[:], tfidx.ap())
            XT = accp.tile([L, BPC], f32)

            def xt_gather(b):
                nc.gpsimd.indirect_dma_start(
                    out=XT[:, b : b + 1],
                    out_offset=None,
                    in_=logits_flat,
                    in_offset=bass.IndirectOffsetOnAxis(ap=TF[:, b : b + 1], axis=0),
                )

            # cols 0-5 fill the Pool queue before the first adj gather is
            # ready; cols 6-7 go after the last adj gather (they are only
            # needed for the CE sum late in the kernel)
            for b in range(6):
                xt_gather(b)

            S = accp.tile([L, BPC], f32)
            M8 = accp.tile([L, BPC * 8], fp16)
            W = accp.tile([L, BPC], f32)  # rows 0..126 hold path weights

            def batch(b):
                X = xp.tile([L, N], f32, tag="X")
                nc.sync.dma_start(X[:], logits_ap[b])
                E = ep.tile([L, N], fp16, tag="E")
                nc.scalar.activation(E[:], X[:], AF.Exp, accum_out=S[:, b : b + 1])
                nc.vector.max(M8[:, 8 * b : 8 * b + 8], E[:])
                i8 = sp.tile([L, 8], u16, tag="i8")
                nc.vector.max_index(i8[:], M8[:, 8 * b : 8 * b + 8], E[:])

                # pair[l] = 1024*idx[l] + idx[l+1] via one PE matmul
                idxf = sp.tile([L, 1], f32, tag="idxf")
                nc.vector.tensor_copy(idxf[:], i8[:, 0:1])
                pairp = pp.tile([L, 1], f32)
                nc.tensor.matmul(
                    out=pairp[:], lhsT=PM[:], rhs=idxf[:], start=True, stop=True
                )
                pairu = sp.tile([L, 1], i32, tag="pairu")
                nc.vector.tensor_copy(pairu[0 : L - 1, :], pairp[0 : L - 1, :])
                nc.gpsimd.indirect_dma_start(
                    out=W[0 : L - 1, b : b + 1],
                    out_offset=None,
                    in_=adjt[b].ap(),
                    in_offset=bass.IndirectOffsetOnAxis(
                        ap=pairu[0 : L - 1, :], axis=0
                    ),
                )

            for b in range(BPC):
                with tc.tile_wait_until(0.0024 * b):
                    batch(b)

            with tc.tile_wait_until(0.0185):
                xt_gather(6)
                xt_gather(7)

            with tc.tile_wait_until(0.0195):
                # cross-entropy partial: sum(ln S - x_t)
                LSE = accp.tile([L, BPC], f32)
                nc.scalar.activation(LSE[:], S[:], AF.Ln)
                R = accp.tile([L, 2], f32)
                nc.vector.memset(R[:, 1:2], 0.0)
                NLL = accp.tile([L, BPC], f32)
                nc.vector.tensor_sub(NLL[:], LSE[:], XT[:])
                nc.vector.reduce_sum(R[:, 0:1], NLL[:], axis=AX.X)

            with tc.tile_wait_until(0.021):
                # balance partial: |per-batch path sums| via PE
                ps_b = pp.tile([BPC, 1], f32)
                nc.tensor.matmul(
                    out=ps_b[:], lhsT=W[0 : L - 1, :], rhs=ones[0 : L - 1, :],
                    start=True, stop=True,
                )
                nc.scalar.activation(R[0:BPC, 1:2], ps_b[:], AF.Abs)

                ps2 = pp.tile([2, 1], f32)
                nc.tensor.matmul(out=ps2[:], lhsT=R[:], rhs=ones[:], start=True, stop=True)
                c2 = sp.tile([2, 1], f32, tag="c2")
                nc.vector.tensor_copy(c2[:], ps2[:])
                nc.scalar.dma_start(out.ap(), c2[:])

    nc.compile()
    return nc


def _get_nc():
    if "nc" not in _CACHE:
        _CACHE["nc"] = _build()
    return _CACHE["nc"]


def _consts():
    if "consts" in _CACHE:
        return _CACHE["consts"]
    ls = np.arange(128)
    pairmm = 1024.0 * (ls[:, None] == ls[None, :]).astype(np.float32) + (
        ls[:, None] == ls[None, :] + 1
    ).astype(np.float32)
    _CACHE["consts"] = pairmm
    return _CACHE["consts"]


def make_in_maps(path_logits, target_paths, adj_matrix):
    """Shard full inputs into per-core in_maps (host-side packing only)."""
    pairmm = _consts()
    l_off = np.arange(L, dtype=np.int64) * N
    b_off = np.arange(BPC, dtype=np.int64)[:, None] * (L * N)
    in_maps = []
    for c in range(NCORES):
        sl = slice(c * BPC, (c + 1) * BPC)
        lg = np.ascontiguousarray(path_logits[sl], dtype=np.float32)
        ads = {
            f"adj{b}": np.ascontiguousarray(
                adj_matrix[c * BPC + b], dtype=np.float32
            ).reshape(N * N, 1)
            for b in range(BPC)
        }
        t = np.asarray(target_paths[sl], dtype=np.int64)
        tf = (b_off + l_off[None, :] + t).astype(np.int32)
        in_maps.append(
            {
                "logits": lg,
                "tfidx": np.ascontiguousarray(tf.T),
                "pairm": pairmm,
                **ads,
            }
        )
    return in_maps


def kernel(**inputs):
    from concourse import bass_utils

    nc = _get_nc()
    in_maps = make_in_maps(
        inputs["path_logits"], inputs["target_paths"], inputs["adj_matrix"]
    )
    res = bass_utils.run_bass_kernel_spmd(nc, in_maps, core_ids=list(range(NCORES)))
    w_nll = np.float32((1.0 - ALPHA) / (B * L))
    w_bal = np.float32(ALPHA / B)
    total = np.float32(0.0)
    for r in res.results:
        total = total + w_nll * np.float32(r["out"][0, 0]) + w_bal * np.float32(
            r["out"][1, 0]
        )
    return np.asarray(total, dtype=np.float32)



# revision 8
# speedup vs baseline: 1.0336x; 1.0336x over previous
"""Trainium2 Bass kernel for CycleBalanceLoss (v2).

loss = ALPHA * mean_b |sum_l adj[b, a_l, a_{l+1}]|        (a = argmax_n logits)
     + (1-ALPHA) * mean_{b,l} (ln sum_n exp(x) - x_target)

Sharding: pure data parallel over batch B=64 across 8 cores (BPC=8).
Host sums the 8 per-core partial scalars.

Key layout/algorithm choices (vs the 53us baseline whose GPSIMD queue was
saturated by 16 indirect DMAs and whose DVE did 2 full 1x scans per batch):

- Host ROTATES each logits row by its target index:
      lgrot[b, l, n] = lg[b, l, (n + t[b,l]) % N]
  so the target logit sits at column 0 of every row and is extracted with a
  61ns DVE copy instead of a ~1.1us indirect DMA per batch (8 fewer SWDGE
  ops on the GPSIMD queue).  The rotated argmax r relates to the true
  argmax by a = (r + t) mod N.
- The mod is made free by uploading a doubled adjacency ADJ2[2048, 2048]
  (ADJ2[u, v] = adj[u & 1023, v & 1023]); gather offset is then simply
  2048*(r+t)_l + (r+t)_{l+1} with (r+t) in [0, 2046].
- argmax per row via prefix-max + count (exact first-max semantics):
      CM = cummax(E)            tensor_tensor_scan op0=max   (1x, ~1.13us)
      idx = sum(CM < CM[-1])    tensor_scalar is_lt + accum  (4x, ~0.33us)
  replacing MAX8 + FIND_INDEX8 (2 x ~1.22us full scans).
- Pair offsets built with one partition-shifted scalar_tensor_tensor
  (2048*ia_l + ia_{l+1}) -- no PE matmul / PSUM copy on the critical chain.
- One indirect DMA per batch (adj gather, 128 offsets incl. a padded row
  127 that always reads offset 0 and is excluded from the balance sum).
"""

import numpy as np

B, L, N = 64, 128, 1024
N2 = 2 * N
NCORES = 8
BPC = B // NCORES
ALPHA = 0.7

_CACHE = {}


def _build():
    import concourse.bacc as bacc
    import concourse.tile as tile
    from concourse import bass, mybir

    f32 = mybir.dt.float32
    fp16 = mybir.dt.float16
    i32 = mybir.dt.int32
    AF = mybir.ActivationFunctionType
    Alu = mybir.AluOpType
    AX = mybir.AxisListType

    nc = bacc.Bacc(
        "TRN2",
        target_bir_lowering=False,
        debug=False,
        num_devices=NCORES,
    )

    logits = nc.dram_tensor("logits", [BPC, L, N], f32, kind="ExternalInput")
    tp2 = nc.dram_tensor("tp2", [L, BPC], f32, kind="ExternalInput")
    pairm = nc.dram_tensor("pairm", [128, 128], f32, kind="ExternalInput")
    adjt = [
        nc.dram_tensor(f"adj{b}", [N2 * N2, 1], f32, kind="ExternalInput")
        for b in range(BPC)
    ]
    out = nc.dram_tensor("out", [2, 1], f32, kind="ExternalOutput")

    logits_ap = logits.ap()

    with tile.TileContext(nc) as tc:
        with (
            tc.tile_pool(name="acc", bufs=1) as accp,
            tc.tile_pool(name="ep", bufs=2) as ep,
            tc.tile_pool(name="cp", bufs=2) as cp,
            tc.tile_pool(name="psum", bufs=2, space="PSUM") as pp,
        ):
            # big input staging: one tile, 8 batch slabs of [128, 1024] f32
            XB = accp.tile([L, BPC * N], f32)
            ones = accp.tile([L, 1], f32)
            TP = accp.tile([L, BPC], f32)    # host const: 2048*t_l + t_{l+1}
            PM = accp.tile([128, 128], f32)  # host const: 2048*I + subdiag(1)
            S = accp.tile([L, BPC], f32)     # per-row sum(exp)
            XT = accp.tile([L, BPC], f32)    # target logits (column 0 of slabs)
            IDX = accp.tile([L, BPC], f32)   # rotated argmax (count result)
            PAIR = accp.tile([L, BPC], i32)  # gather offsets
            W = accp.tile([L, BPC], f32)     # gathered path weights
            M32 = accp.tile([L, 1], f32)     # row max as f32 scalar ap
            DUM = accp.tile([L, N], fp16)    # scratch elementwise output
            LSE = accp.tile([L, BPC], f32)
            NLLt = accp.tile([L, BPC], f32)
            R = accp.tile([L, 2], f32)
            TINY = accp.tile([L, 1], f32)

            # stream all logits slabs up front (no deps; one HWDGE queue)
            for b in range(BPC):
                nc.sync.dma_start(XB[:, b * N : (b + 1) * N], logits_ap[b])
            nc.scalar.dma_start(TP[:], tp2.ap())
            nc.scalar.dma_start(PM[:], pairm.ap())

            nc.vector.memset(ones[:], 1.0)
            nc.vector.memset(R[:, 1:2], 0.0)

            # prime the Ln activation table early so the scheduler doesn't
            # stall the ACT queue mid-stream with a 1.3us table load
            nc.scalar.activation(TINY[:], ones[:], AF.Ln)

            for b in range(BPC):
                Xb = XB[:, b * N : (b + 1) * N]
                E = ep.tile([L, N], fp16, tag="E")
                nc.scalar.activation(E[:], Xb, AF.Exp, accum_out=S[:, b : b + 1])

                CM = cp.tile([L, N], fp16, tag="CM")
                nc.vector.tensor_tensor_scan(
                    CM[:], E[:], E[:], 0.0, op0=Alu.max, op1=Alu.bypass
                )
                nc.vector.tensor_copy(M32[:], CM[:, N - 1 : N])
                nc.vector.tensor_scalar(
                    DUM[:],
                    CM[:],
                    M32[:],
                    None,
                    Alu.is_lt,
                    Alu.add,
                    accum_out=IDX[:, b : b + 1],
                )
                # pair_l = 2048*r_l + r_{l+1} via PE (partition shift is the
                # subdiag of PM; engines cannot read partition-offset APs),
                # then +(2048*t_l + t_{l+1}) host constant during PSUM copy
                pairp = pp.tile([L, 1], f32)
                nc.tensor.matmul(
                    out=pairp[:], lhsT=PM[:], rhs=IDX[:, b : b + 1],
                    start=True, stop=True,
                )
                nc.vector.scalar_tensor_tensor(
                    PAIR[:, b : b + 1],
                    pairp[:],
                    1.0,
                    TP[:, b : b + 1],
                    op0=Alu.mult,
                    op1=Alu.add,
                )
                nc.vector.tensor_copy(XT[:, b : b + 1], Xb[:, 0:1])
                nc.gpsimd.indirect_dma_start(
                    out=W[:, b : b + 1],
                    out_offset=None,
                    in_=adjt[b].ap(),
                    in_offset=bass.IndirectOffsetOnAxis(
                        ap=PAIR[:, b : b + 1], axis=0
                    ),
                )

            # cross-entropy partial: R[:,0] = sum_b (ln S - x_t)
            nc.scalar.activation(LSE[:], S[:], AF.Ln)
            nc.vector.tensor_sub(NLLt[:], LSE[:], XT[:])
            nc.vector.reduce_sum(R[:, 0:1], NLLt[:], axis=AX.X)

            # balance partial: |per-batch path sums| via PE
            psB = pp.tile([BPC, 1], f32)
            nc.tensor.matmul(
                out=psB[:],
                lhsT=W[0 : L - 1, :],
                rhs=ones[0 : L - 1, :],
                start=True,
                stop=True,
            )
            nc.scalar.activation(R[0:BPC, 1:2], psB[:], AF.Abs)

            ps2 = pp.tile([2, 1], f32)
            nc.tensor.matmul(out=ps2[:], lhsT=R[:], rhs=ones[:], start=True, stop=True)
            c2 = accp.tile([2, 1], f32)
            nc.vector.tensor_copy(c2[:], ps2[:])
            nc.scalar.dma_start(out.ap(), c2[:])

    nc.compile()
    return nc


def _get_nc():
    if "nc" not in _CACHE:
        _CACHE["nc"] = _build()
    return _CACHE["nc"]


def _pairm():
    if "pairm" not in _CACHE:
        ls = np.arange(128)
        pm = float(N2) * (ls[:, None] == ls[None, :]).astype(np.float32) + (
            ls[:, None] == ls[None, :] + 1
        ).astype(np.float32)
        _CACHE["pairm"] = pm
    return _CACHE["pairm"]


def make_in_maps(path_logits, target_paths, adj_matrix):
    """Shard + repack full inputs into per-core in_maps (host-side only)."""
    ar = np.arange(N, dtype=np.int64)
    pm = _pairm()
    in_maps = []
    for c in range(NCORES):
        sl = slice(c * BPC, (c + 1) * BPC)
        lg = np.asarray(path_logits[sl], dtype=np.float32)
        t = np.asarray(target_paths[sl], dtype=np.int64)  # [BPC, L]
        # rotate each row so the target logit is at column 0
        rot = (ar[None, None, :] + t[:, :, None]) % N
        lgrot = np.ascontiguousarray(np.take_along_axis(lg, rot, axis=2))
        # host constant: 2048*t_l + t_{l+1} (last row: 2048*t_127)
        tp = float(N2) * t.astype(np.float64)
        tp[:, : L - 1] += t[:, 1:].astype(np.float64)
        ads = {}
        for b in range(BPC):
            a = np.asarray(adj_matrix[c * BPC + b], dtype=np.float32)
            ads[f"adj{b}"] = np.ascontiguousarray(
                np.tile(a, (2, 2))
            ).reshape(N2 * N2, 1)
        in_maps.append(
            {
                "logits": lgrot,
                "tp2": np.ascontiguousarray(tp.T.astype(np.float32)),
                "pairm": pm,
                **ads,
            }
        )
    return in_maps


def kernel(**inputs):
    from concourse import bass_utils

    nc = _get_nc()
    in_maps = make_in_maps(
        inputs["path_logits"], inputs["target_paths"], inputs["adj_matrix"]
    )
    res = bass_utils.run_bass_kernel_spmd(nc, in_maps, core_ids=list(range(NCORES)))
    w_nll = np.float32((1.0 - ALPHA) / (B * L))
    w_bal = np.float32(ALPHA / B)
    total = np.float32(0.0)
    for r in res.results:
        total = total + w_nll * np.float32(r["out"][0, 0]) + w_bal * np.float32(
            r["out"][1, 0]
        )
    return np.asarray(total, dtype=np.float32)


# revision 12
# speedup vs baseline: 1.2448x; 1.2043x over previous
"""Trainium2 Bass kernel for CycleBalanceLoss (v2).

loss = ALPHA * mean_b |sum_l adj[b, a_l, a_{l+1}]|        (a = argmax_n logits)
     + (1-ALPHA) * mean_{b,l} (ln sum_n exp(x) - x_target)

Sharding: pure data parallel over batch B=64 across 8 cores (BPC=8).
Host sums the 8 per-core partial scalars.

Key layout/algorithm choices (vs the 53us baseline whose GPSIMD queue was
saturated by 16 indirect DMAs and whose DVE did 2 full 1x scans per batch):

- Host ROTATES each logits row by its target index:
      lgrot[b, l, n] = lg[b, l, (n + t[b,l]) % N]
  so the target logit sits at column 0 of every row and is extracted with a
  61ns DVE copy instead of a ~1.1us indirect DMA per batch (8 fewer SWDGE
  ops on the GPSIMD queue).  The rotated argmax r relates to the true
  argmax by a = (r + t) mod N.
- The mod is made free by uploading a doubled adjacency ADJ2[2048, 2048]
  (ADJ2[u, v] = adj[u & 1023, v & 1023]); gather offset is then simply
  2048*(r+t)_l + (r+t)_{l+1} with (r+t) in [0, 2046].
- argmax per row via prefix-max + count (exact first-max semantics):
      CM = cummax(E)            tensor_tensor_scan op0=max   (1x, ~1.13us)
      idx = sum(CM < CM[-1])    tensor_scalar is_lt + accum  (4x, ~0.33us)
  replacing MAX8 + FIND_INDEX8 (2 x ~1.22us full scans).
- Pair offsets built with one partition-shifted scalar_tensor_tensor
  (2048*ia_l + ia_{l+1}) -- no PE matmul / PSUM copy on the critical chain.
- One indirect DMA per batch (adj gather, 128 offsets incl. a padded row
  127 that always reads offset 0 and is excluded from the balance sum).
"""

import numpy as np

B, L, N = 64, 128, 1024
N2 = 2 * N
NCORES = 8
BPC = B // NCORES
ALPHA = 0.7

_CACHE = {}


def _build():
    import concourse.bacc as bacc
    import concourse.tile as tile
    from concourse import bass, mybir

    f32 = mybir.dt.float32
    fp16 = mybir.dt.float16
    i32 = mybir.dt.int32
    u16 = mybir.dt.uint16
    AF = mybir.ActivationFunctionType
    Alu = mybir.AluOpType
    AX = mybir.AxisListType

    nc = bacc.Bacc(
        "TRN2",
        target_bir_lowering=False,
        debug=False,
        num_devices=NCORES,
    )

    logits = nc.dram_tensor("logits", [BPC, L, N], f32, kind="ExternalInput")
    tp2 = nc.dram_tensor("tp2", [L, BPC], f32, kind="ExternalInput")
    pairm = nc.dram_tensor("pairm", [128, 128], f32, kind="ExternalInput")
    adjt = [
        nc.dram_tensor(f"adj{b}", [N2 * N2, 1], f32, kind="ExternalInput")
        for b in range(BPC)
    ]
    out = nc.dram_tensor("out", [2, 1], f32, kind="ExternalOutput")

    logits_ap = logits.ap()

    with tile.TileContext(nc) as tc:
        with (
            tc.tile_pool(name="acc", bufs=1) as accp,
            tc.tile_pool(name="ep", bufs=2) as ep,
            tc.tile_pool(name="cp", bufs=2) as cp,
            tc.tile_pool(name="psum", bufs=2, space="PSUM") as pp,
        ):
            # big input staging: one tile, 8 batch slabs of [128, 1024] f32
            XB = accp.tile([L, BPC * N], f32)
            ones = accp.tile([L, 1], f32)
            TP = accp.tile([L, BPC], f32)    # host const: 2048*t_l + t_{l+1}
            PM = accp.tile([128, 128], f32)  # host const: 2048*I + subdiag(1)
            S = accp.tile([L, BPC], f32)     # per-row sum(exp)
            XT = accp.tile([L, BPC], f32)    # target logits (column 0 of slabs)
            M8 = accp.tile([L, BPC * 8], fp16)  # top-8 values per batch
            IDX = accp.tile([L, BPC], f32)   # rotated argmax as f32
            PAIR = accp.tile([L, BPC], i32)  # gather offsets
            W = accp.tile([L, BPC], f32)     # gathered path weights
            LSE = accp.tile([L, BPC], f32)
            NLLt = accp.tile([L, BPC], f32)
            R = accp.tile([L, 2], f32)
            TINY = accp.tile([L, 1], f32)

            # stream all logits slabs up front (no deps; one HWDGE queue)
            for b in range(BPC):
                nc.sync.dma_start(XB[:, b * N : (b + 1) * N], logits_ap[b])
            nc.scalar.dma_start(TP[:], tp2.ap())
            nc.scalar.dma_start(PM[:], pairm.ap())

            nc.vector.memset(ones[:], 1.0)
            nc.vector.memset(R[:, 1:2], 0.0)

            # prime the Ln activation table early so the scheduler doesn't
            # stall the ACT queue mid-stream with a 1.3us table load
            nc.scalar.activation(TINY[:], ones[:], AF.Ln)

            for b in range(BPC):
                Xb = XB[:, b * N : (b + 1) * N]
                E = ep.tile([L, N], fp16, tag="E")
                nc.scalar.activation(E[:], Xb, AF.Exp, accum_out=S[:, b : b + 1])

                nc.vector.max(M8[:, 8 * b : 8 * b + 8], E[:])
                i8 = cp.tile([L, 8], u16, tag="i8")
                nc.vector.max_index(i8[:], M8[:, 8 * b : 8 * b + 8], E[:])
                nc.vector.tensor_copy(IDX[:, b : b + 1], i8[:, 0:1])

                # pair_l = 2048*r_l + r_{l+1} via PE (partition shift is the
                # subdiag of PM; engines cannot read partition-offset APs),
                # then +(2048*t_l + t_{l+1}) host constant during PSUM copy
                pairp = pp.tile([L, 1], f32)
                nc.tensor.matmul(
                    out=pairp[:], lhsT=PM[:], rhs=IDX[:, b : b + 1],
                    start=True, stop=True,
                )
                nc.vector.scalar_tensor_tensor(
                    PAIR[:, b : b + 1],
                    pairp[:],
                    1.0,
                    TP[:, b : b + 1],
                    op0=Alu.mult,
                    op1=Alu.add,
                )
                nc.gpsimd.indirect_dma_start(
                    out=W[:, b : b + 1],
                    out_offset=None,
                    in_=adjt[b].ap(),
                    in_offset=bass.IndirectOffsetOnAxis(
                        ap=PAIR[:, b : b + 1], axis=0
                    ),
                )

            # target logits: one strided copy of column 0 of every slab
            nc.vector.tensor_copy(XT[:], XB[:].rearrange("p (b n) -> p b n", b=BPC)[:, :, 0:1])

            # cross-entropy partial: R[:,0] = sum_b (ln S - x_t)
            nc.scalar.activation(LSE[:], S[:], AF.Ln)
            nc.vector.tensor_sub(NLLt[:], LSE[:], XT[:])
            nc.vector.reduce_sum(R[:, 0:1], NLLt[:], axis=AX.X)

            # balance partial: |per-batch path sums| via PE; abs on DVE
            # (max(x, -x)) to avoid a third activation-table load
            psB = pp.tile([BPC, 1], f32)
            nc.tensor.matmul(
                out=psB[:],
                lhsT=W[0 : L - 1, :],
                rhs=ones[0 : L - 1, :],
                start=True,
                stop=True,
            )
            nc.vector.tensor_reduce(
                R[0:BPC, 1:2], psB[:], axis=AX.X, op=Alu.max,
                apply_absolute_value=True,
            )

            ps2 = pp.tile([2, 1], f32)
            nc.tensor.matmul(out=ps2[:], lhsT=R[:], rhs=ones[:], start=True, stop=True)
            c2 = accp.tile([2, 1], f32)
            nc.vector.tensor_copy(c2[:], ps2[:])
            nc.scalar.dma_start(out.ap(), c2[:])

    nc.compile()
    return nc


def _get_nc():
    if "nc" not in _CACHE:
        _CACHE["nc"] = _build()
    return _CACHE["nc"]


def _pairm():
    if "pairm" not in _CACHE:
        ls = np.arange(128)
        pm = float(N2) * (ls[:, None] == ls[None, :]).astype(np.float32) + (
            ls[:, None] == ls[None, :] + 1
        ).astype(np.float32)
        _CACHE["pairm"] = pm
    return _CACHE["pairm"]


def make_in_maps(path_logits, target_paths, adj_matrix):
    """Shard + repack full inputs into per-core in_maps (host-side only)."""
    ar = np.arange(N, dtype=np.int64)
    pm = _pairm()
    in_maps = []
    for c in range(NCORES):
        sl = slice(c * BPC, (c + 1) * BPC)
        lg = np.asarray(path_logits[sl], dtype=np.float32)
        t = np.asarray(target_paths[sl], dtype=np.int64)  # [BPC, L]
        # rotate each row so the target logit is at column 0
        rot = (ar[None, None, :] + t[:, :, None]) % N
        lgrot = np.ascontiguousarray(np.take_along_axis(lg, rot, axis=2))
        # host constant: 2048*t_l + t_{l+1} (last row: 2048*t_127)
        tp = float(N2) * t.astype(np.float64)
        tp[:, : L - 1] += t[:, 1:].astype(np.float64)
        ads = {}
        for b in range(BPC):
            a = np.asarray(adj_matrix[c * BPC + b], dtype=np.float32)
            ads[f"adj{b}"] = np.ascontiguousarray(
                np.tile(a, (2, 2))
            ).reshape(N2 * N2, 1)
        in_maps.append(
            {
                "logits": lgrot,
                "tp2": np.ascontiguousarray(tp.T.astype(np.float32)),
                "pairm": pm,
                **ads,
            }
        )
    return in_maps


def kernel(**inputs):
    from concourse import bass_utils

    nc = _get_nc()
    in_maps = make_in_maps(
        inputs["path_logits"], inputs["target_paths"], inputs["adj_matrix"]
    )
    res = bass_utils.run_bass_kernel_spmd(nc, in_maps, core_ids=list(range(NCORES)))
    w_nll = np.float32((1.0 - ALPHA) / (B * L))
    w_bal = np.float32(ALPHA / B)
    total = np.float32(0.0)
    for r in res.results:
        total = total + w_nll * np.float32(r["out"][0, 0]) + w_bal * np.float32(
            r["out"][1, 0]
        )
    return np.asarray(total, dtype=np.float32)
